# revision 3
# baseline (speedup 1.0000x reference)
"""Causal single-head attention (B=4, S=4096, d=1024) on 8 Trainium2 cores.

Sharding: 8 cores = 4 batches x 2 q-groups.  Per batch, the 16 causal
q-blocks of 256 rows (k-tile coverage 2,4,...,32) are assigned one per
capacity class c=1..8 (capacity 4c) on each core, giving a uniform static
program of 8 slots with capacities (4,8,...,32) = 144 k-tile visits vs 136
real (6% padding).  Causal masking and padding use the data-driven mask
    A = exp(s/32) * (I - J <= delta)
applied only to the last 4 visits of each slot; the final visit of a slot
only touches its upper q-half (the lower half is fully masked), so its
score matmul runs at N=128 and its attn@v skips m=0.

All matmul operands are bf16 (same 1 row/cycle PE rate as fp32r,
half the SBUF), so kT [128,8,4096] and v [128,32,1024] stay SBUF-resident
for the whole program: attention streams NOTHING from DRAM except one qt
tile per slot.  x and W are pre-cast to bf16 on the host.  Accumulation
stays fp32 in PSUM; output is fp32.  Softmax normalization runs on the
otherwise-idle DVE (reciprocal + per-partition scale) and output stores go
out on the DVE DMA queue, keeping ACT free for the exp stream.  Weight /
x-block / q-block loads are spread over the sync, gpsimd and vector queues
and prefetched a phase ahead, so the PE never waits on DMA after startup.

Per-core PE work: kv proj 524k rows + q proj 131k + attention ~574k rows
~ 510 us at 2.4 GHz.
"""

import contextlib
import math

import numpy as np
import ml_dtypes

import concourse.bass as bass  # noqa: F401
import concourse.mybir as mybir
import concourse.tile as tile
from concourse import bacc
from concourse.bass_utils import run_bass_kernel_spmd

F32 = mybir.dt.float32
BF16 = mybir.dt.bfloat16
AF = mybir.ActivationFunctionType
ALU = mybir.AluOpType
NPBF = ml_dtypes.bfloat16

CFG_FULL = dict(S=4096, D=1024, QBLK=256)
# per-batch q-block indices (q0 = 256j) per core half, ascending capacity
QJ_FULL = {0: (0, 3, 4, 7, 8, 11, 12, 15), 1: (1, 2, 5, 6, 9, 10, 13, 14)}
B_FULL = 4


def build_nc(S, D, QBLK, reps=1):
    DC = D // 128
    NSLOT = 8
    CAPS = [4 * (s + 1) for s in range(NSLOT)]
    MAXCOV = max(CAPS)
    QROWS = NSLOT * QBLK
    M = QBLK // 128
    NT = S // 128
    SBLK = 512
    NSB = S // SBLK
    NQB = QROWS // SBLK
    scale = 1.0 / math.sqrt(D)
    assert MAXCOV == NT

    nc = bacc.Bacc("TRN2", target_bir_lowering=False)
    xT_d = nc.dram_tensor("xT", [D, S], BF16, kind="ExternalInput")
    xTq_d = nc.dram_tensor("xTq", [D, QROWS], BF16, kind="ExternalInput")
    wq_d = nc.dram_tensor("Wq", [D, D], BF16, kind="ExternalInput")
    wk_d = nc.dram_tensor("Wk", [D, D], BF16, kind="ExternalInput")
    wv_d = nc.dram_tensor("Wv", [D, D], BF16, kind="ExternalInput")
    ij_d = nc.dram_tensor("IJ", [128, QBLK], F32, kind="ExternalInput")
    dl_d = nc.dram_tensor("delta", [128, NSLOT * MAXCOV], F32,
                          kind="ExternalInput")
    ones_d = nc.dram_tensor("ones", [128, 2], BF16, kind="ExternalInput")
    out_d = nc.dram_tensor("out", [QROWS, D], F32, kind="ExternalOutput")

    def dpart(ap):
        return ap.rearrange("(c p) n -> p c n", p=128)

    with tile.TileContext(nc) as tc:
        with tc.tile_pool(name="dram", bufs=1, space="DRAM") as dram, \
             tc.tile_pool(name="res", bufs=1) as res, \
             tc.tile_pool(name="xp", bufs=2) as xpool, \
             tc.tile_pool(name="wqp", bufs=1) as wqpool, \
             tc.tile_pool(name="qt", bufs=2) as qtpool, \
             tc.tile_pool(name="sm", bufs=1) as smpool, \
             tc.tile_pool(name="dummy", bufs=1, space="PSUM") as dummypool:
            qT_i = dram.tile([DC, 128, QROWS], BF16, name="qT_i")
            kT_sb = res.tile([128, DC, S], BF16, name="kT_sb", tag="kT")
            v_sb = res.tile([128, NT, D], BF16, name="v_sb", tag="v")
            dummy_ps = dummypool.tile([128, 2], F32, name="dummy_ps",
                                      tag="dummy")

            def touch(cols2):
                # Tiny matmul reading a freshly DMA'd SBUF tile so the PE
                # observes the DMA tick with a single sync wait.
                nc.tensor.matmul(dummy_ps[0:1, 0:2], cols2[:, 0:1], cols2,
                                 start=True, stop=True)

            _loop = (tc.For_i(0, reps, 1) if reps > 1
                     else contextlib.nullcontext())
            with _loop:
                def x_load(idx, src=xT_d, queue=None):
                    xt = xpool.tile([128, DC, SBLK], BF16, name="xt",
                                    tag="xt")
                    (queue or nc.sync).dma_start(
                        out=xt,
                        in_=dpart(src[:, idx * SBLK:(idx + 1) * SBLK]))
                    return xt

                # ---------------- Phase 1: k & v projections ----------------
                with (
                    tc.tile_pool(name="wkv", bufs=1) as wpool,
                    tc.tile_pool(name="kps", bufs=2, space="PSUM") as kpsum,
                    tc.tile_pool(name="vps", bufs=2, space="PSUM") as vpsum,
                ):
                    xts = {0: x_load(0, queue=nc.scalar)}
                    w_sb = {}
                    for name, wd, q in (("k", wk_d, nc.sync),
                                        ("v", wv_d, nc.scalar)):
                        w_sb[name] = wpool.tile([128, DC, D], BF16,
                                                name=f"w{name}",
                                                tag=f"w{name}")
                        q.dma_start(out=w_sb[name], in_=dpart(wd[:, :]))
                        touch(w_sb[name][:, 0, 0:2])

                    wq_sb = wqpool.tile([128, DC, D], BF16, name="wq",
                                        tag="wq")
                    nc.scalar.dma_start(out=wq_sb, in_=dpart(wq_d[:, :]))
                    ij_sb = smpool.tile([128, QBLK], F32, name="ij",
                                        tag="ij")
                    nc.scalar.dma_start(out=ij_sb, in_=ij_d[:, :])
                    dl_sb = smpool.tile([128, NSLOT * MAXCOV], F32,
                                        name="dl", tag="dl")
                    nc.scalar.dma_start(out=dl_sb, in_=dl_d[:, :])
                    ones_sb = smpool.tile([128, 2], BF16, name="ones",
                                          tag="ones")
                    nc.scalar.dma_start(out=ones_sb, in_=ones_d[:, :])
                    touch(ones_sb)
                    xtq0 = None
                    for b in range(NSB):
                        if b + 1 < NSB:
                            xts[b + 1] = x_load(b + 1)
                        elif b == NSB - 1:
                            xtq0 = x_load(NQB - 1, src=xTq_d)
                        xt = xts.pop(b)
                        touch(xt[:, 0, 0:2])
                        for co in range(DC):
                            ps = kpsum.tile([128, SBLK], F32, name="kp",
                                            tag="kp")
                            for ci in range(DC):
                                nc.tensor.matmul(
                                    ps,
                                    w_sb["k"][:, ci, co * 128:(co + 1) * 128],
                                    xt[:, ci, :],
                                    start=(ci == 0), stop=(ci == DC - 1))
                            nc.scalar.copy(
                                out=kT_sb[:, co, b * SBLK:(b + 1) * SBLK],
                                in_=ps)
                        for m in range(SBLK // 128):
                            for h in range(2):
                                ps = vpsum.tile([128, D // 2], F32,
                                                name="vp", tag="vp")
                                for ci in range(DC):
                                    nc.tensor.matmul(
                                        ps,
                                        xt[:, ci, m * 128:(m + 1) * 128],
                                        w_sb["v"][:, ci, h * (D // 2):
                                                  (h + 1) * (D // 2)],
                                        start=(ci == 0), stop=(ci == DC - 1))
                                nc.scalar.copy(
                                    out=v_sb[:, b * (SBLK // 128) + m,
                                             h * (D // 2):(h + 1) * (D // 2)],
                                    in_=ps)

                # ---------------- Phase 2: q projection (reversed) ----------
                def qt_load(s, queue):
                    qt = qtpool.tile([128, DC, QBLK], BF16, name="qt",
                                     tag="qt")
                    queue.dma_start(
                        out=qt,
                        in_=qT_i[:, :, s * QBLK:(s + 1) * QBLK]
                        .rearrange("c p y -> p c y"))
                    return qt

                order = list(range(NSLOT - 1, -1, -1))  # big slots first
                qt_next = None
                with (
                    tc.tile_pool(name="qs", bufs=3) as qspool,
                    tc.tile_pool(name="qps", bufs=2, space="PSUM") as qpsum,
                ):
                    touch(wq_sb[:, 0, 0:2])
                    xtqs = {NQB - 1: xtq0}
                    for bi, b in enumerate(range(NQB - 1, -1, -1)):
                        if b - 1 >= 0:
                            xtqs[b - 1] = x_load(b - 1, src=xTq_d)
                        xtq = xtqs.pop(b)
                        touch(xtq[:, 0, 0:2])
                        for co in range(DC):
                            ps = qpsum.tile([128, SBLK], F32, name="qp",
                                            tag="qp")
                            for ci in range(DC):
                                nc.tensor.matmul(
                                    ps,
                                    wq_sb[:, ci, co * 128:(co + 1) * 128],
                                    xtq[:, ci, :],
                                    start=(ci == 0), stop=(ci == DC - 1))
                            qs = qspool.tile([128, SBLK], BF16, name="qs",
                                             tag="qs")
                            nc.scalar.copy(out=qs, in_=ps)
                            nc.scalar.dma_start(
                                out=qT_i[co, :, b * SBLK:(b + 1) * SBLK],
                                in_=qs)
                        if b == NQB - 1:
                            qt_next = qt_load(order[0], nc.sync)

                # ---------------- Phase 3: attention ----------------
                with (
                    tc.tile_pool(name="at", bufs=4) as apool,
                    tc.tile_pool(name="cm", bufs=2) as cmpool,
                    tc.tile_pool(name="ot", bufs=4) as otpool,
                    tc.tile_pool(name="rc", bufs=2) as rcpool,
                    tc.tile_pool(name="sps", bufs=2, space="PSUM") as spsum,
                    tc.tile_pool(name="ops", bufs=2 * M, space="PSUM") as opsum,
                    tc.tile_pool(name="dps", bufs=1, space="PSUM") as dpsum,
                ):
                    for oi, s in enumerate(order):
                        cap = CAPS[s]
                        qt, qt_next = qt_next, None
                        touch(qt[:, 0, 0:2])
                        if oi + 1 < len(order):
                            qt_next = qt_load(order[oi + 1], nc.sync)
                        po = [opsum.tile([128, D // 2], F32, name="po",
                                         tag="po") for _ in range(2 * M)]
                        pd = dpsum.tile([128, 2 * M], F32, name="pd",
                                        tag="pd")

                        def attnv(i, at, first, last):
                            # final visit: lower q-half fully masked, so
                            # m=0 is skipped there and closes one visit early
                            for m in range((1 if last and cap > 1 else 0), M):
                                mlast = cap - 1 if (m == 1 or cap == 1) \
                                    else cap - 2
                                for h in range(2):
                                    nc.tensor.matmul(
                                        po[2 * m + h],
                                        at[:, m * 128:(m + 1) * 128],
                                        v_sb[:, i, h * (D // 2):
                                             (h + 1) * (D // 2)],
                                        start=first, stop=(i == mlast))
                                nc.tensor.matmul(
                                    pd[:, 2 * m:2 * m + 2],
                                    at[:, m * 128:(m + 1) * 128],
                                    ones_sb[:, :],
                                    start=(first and m == 0),
                                    stop=(i == cap - 1 and m == M - 1))

                        pend = []  # (visit, at) awaiting attn@v, depth 2
                        for i in range(cap):
                            lastv = (i == cap - 1 and cap > 1)
                            lo = 128 if lastv else 0  # final visit: N=128
                            ps = spsum.tile([128, QBLK], F32, name="ps",
                                            tag="ps")
                            for ci in range(DC):
                                nc.tensor.matmul(
                                    ps[:, lo:],
                                    kT_sb[:, ci, i * 128:(i + 1) * 128],
                                    qt[:, ci, lo:],
                                    start=(ci == 0), stop=(ci == DC - 1))
                            at = apool.tile([128, QBLK], BF16, name="at",
                                            tag="at")
                            nc.scalar.activation(
                                out=at[:, lo:], in_=ps[:, lo:], func=AF.Exp,
                                scale=scale)
                            if i >= cap - 4:
                                cm = cmpool.tile([128, QBLK], BF16,
                                                 name="cm", tag="cm")
                                nc.vector.tensor_scalar(
                                    out=cm[:, lo:], in0=ij_sb[:, lo:],
                                    scalar1=dl_sb[:, s * MAXCOV + i:
                                                  s * MAXCOV + i + 1],
                                    scalar2=None, op0=ALU.is_le)
                                nc.vector.tensor_mul(
                                    out=at[:, lo:], in0=at[:, lo:],
                                    in1=cm[:, lo:])
                            pend.append((i, at))
                            if len(pend) > 2:
                                j, aj = pend.pop(0)
                                attnv(j, aj, j == 0, False)
                        for j, aj in pend:
                            attnv(j, aj, j == 0, j == cap - 1)

                        rc = rcpool.tile([128, 2 * M], F32, name="rc",
                                         tag="rc")
                        nc.vector.reciprocal(out=rc, in_=pd)
                        for m in range(M):
                            for h in range(2):
                                ot = otpool.tile([128, D // 2], F32,
                                                 name="ot", tag="ot")
                                nc.vector.tensor_scalar(
                                    out=ot, in0=po[2 * m + h],
                                    scalar1=rc[:, 2 * m:2 * m + 1],
                                    scalar2=None, op0=ALU.mult)
                                oq = (nc.scalar if (oi == len(order) - 1
                                                    and m == 0)
                                      else nc.sync)
                                oq.dma_start(
                                    out=out_d[s * QBLK + m * 128:
                                              s * QBLK + (m + 1) * 128,
                                              h * (D // 2):(h + 1) * (D // 2)],
                                    in_=ot)
    nc.compile()
    return nc


def host_core_inputs(x_b, Wq, Wk, Wv, qjs, S, D, QBLK):
    NSLOT = len(qjs)
    MAXCOV = S // 128
    xT = np.ascontiguousarray(x_b.T.astype(NPBF))
    xTq = np.ascontiguousarray(np.concatenate(
        [x_b[j * QBLK:(j + 1) * QBLK] for j in qjs], axis=0).T.astype(NPBF))
    ij = (np.arange(128, dtype=np.float32)[:, None]
          - np.arange(QBLK, dtype=np.float32)[None, :])
    ij = np.ascontiguousarray(np.broadcast_to(ij, (128, QBLK)))
    delta = np.empty((NSLOT, MAXCOV), dtype=np.float32)
    for s, j in enumerate(qjs):
        delta[s, :] = j * QBLK - 128.0 * np.arange(MAXCOV, dtype=np.float32)
    delta = np.ascontiguousarray(
        np.broadcast_to(delta.reshape(1, -1), (128, NSLOT * MAXCOV)))
    return {
        "xT": xT, "xTq": xTq,
        "Wq": np.ascontiguousarray(Wq.astype(NPBF)),
        "Wk": np.ascontiguousarray(Wk.astype(NPBF)),
        "Wv": np.ascontiguousarray(Wv.astype(NPBF)),
        "IJ": ij, "delta": delta,
        "ones": np.ones((128, 2), NPBF),
    }


_NC_CACHE = {}


def _get_nc(key, cfg, **kw):
    if key not in _NC_CACHE:
        _NC_CACHE[key] = build_nc(**cfg, **kw)
    return _NC_CACHE[key]


def run_full(x, Wq, Wk, Wv, trace=False, trace_cores=None):
    cfg = CFG_FULL
    S, D, QBLK = cfg["S"], cfg["D"], cfg["QBLK"]
    x = np.asarray(x, np.float32)
    Wq = np.asarray(Wq, np.float32)
    Wk = np.asarray(Wk, np.float32)
    Wv = np.asarray(Wv, np.float32)
    B = x.shape[0]
    assert (B, x.shape[1], x.shape[2]) == (B_FULL, S, D)

    nc = _get_nc("full", cfg)
    in_maps = []
    for b in range(B):
        for h in range(2):
            in_maps.append(host_core_inputs(
                x[b], Wq, Wk, Wv, QJ_FULL[h], S, D, QBLK))
    res = run_bass_kernel_spmd(
        nc, in_maps, list(range(2 * B)), trace=trace,
        trace_cores=trace_cores)
    out = np.empty((B, S, D), np.float32)
    for b in range(B):
        for h in range(2):
            o = np.asarray(res.results[2 * b + h]["out"])
            for s, j in enumerate(QJ_FULL[h]):
                out[b, j * QBLK:(j + 1) * QBLK] = o[s * QBLK:(s + 1) * QBLK]
    return out, res


def kernel(x, Wq, Wk, Wv):
    out, _ = run_full(x, Wq, Wk, Wv)
    return out
